# revision 1
# baseline (speedup 1.0000x reference)
"""Trainium2 Bass kernel for DeterministicActorRNN.

Network (per sample):
  obs   = state[:, :1280] -> 5 frames of 256, time-reversed
  2-layer tanh RNN (H=256, T=5)  -> last hidden of layer 2
  MLP: relu(320->1024), relu(1024->1024), 64 <- tanh(1024->64)

Strategy: pure data parallel over 8 NeuronCores (2048 rows each).
All activations are kept feature-major ([feature, batch]) so every matmul
contracts the partition dimension; the host pre-transposes the state slice
and all weight matrices.  Matmuls run in float32r (fp32 storage, 11-bit
mantissa multiply) at full PE rate; PSUM accumulation and all bias/tanh/relu
math stay fp32.  Each core processes its 2048 rows as 2 independent
batch-chunks of 1024 to bound SBUF usage and give the scheduler two
independent RNN chains to interleave.
"""

import numpy as np

B = 16384
NCORES = 8
BL = B // NCORES          # 2048 rows per core
NPASS = 2
BP = BL // NPASS          # 1024 rows per pass
T = 5
H = 256                   # RNN hidden / obs per frame
G = 64                    # goal dim
OBS = T * H               # 1280
HID = 1024
OUT = 64
NKH = H // 128            # 2 feature blocks of 128

_EXEC = None              # compiled executable cache


def _build_bass():
    import os
    import concourse.bass as bass  # noqa: F401
    from concourse import bacc
    import concourse.mybir as mybir
    from concourse.tile import TileContext

    K_T = int(os.environ.get("K_T", T))          # RNN steps to run
    K_MLP = int(os.environ.get("K_MLP", "1"))    # include MLP
    K_RNN = int(os.environ.get("K_RNN", "1"))    # include RNN
    K_REPS = int(os.environ.get("K_REPS", "1"))  # repeat whole body (timing)
    XB = int(os.environ.get("K_XB", "6"))        # x pool bufs
    HB = int(os.environ.get("K_HB", "10"))       # h pool bufs
    MB = int(os.environ.get("K_MB", "16"))       # mlp pool bufs
    PB = int(os.environ.get("K_PB", "3"))        # psum pool bufs

    F32 = mybir.dt.float32
    F32R = mybir.dt.float32r
    TANH = mybir.ActivationFunctionType.Tanh
    RELU = mybir.ActivationFunctionType.Relu

    nc = bacc.Bacc(None, target_bir_lowering=False)

    xT = nc.dram_tensor("xT", [OBS + G, BL], F32R, kind="ExternalInput")
    wi_d = [nc.dram_tensor(f"wi{l}T", [H, H], F32R, kind="ExternalInput") for l in range(2)]
    wh_d = [nc.dram_tensor(f"wh{l}T", [H, H], F32R, kind="ExternalInput") for l in range(2)]
    brnn_d = [nc.dram_tensor(f"brnn{l}", [128, NKH], F32, kind="ExternalInput") for l in range(2)]
    w1_d = nc.dram_tensor("w1T", [H + G, HID], F32R, kind="ExternalInput")
    w2_d = nc.dram_tensor("w2T", [HID, HID], F32R, kind="ExternalInput")
    w3_d = nc.dram_tensor("w3T", [HID, OUT], F32R, kind="ExternalInput")
    b1_d = nc.dram_tensor("b1v", [128, 8], F32, kind="ExternalInput")
    b2_d = nc.dram_tensor("b2v", [128, 8], F32, kind="ExternalInput")
    b3_d = nc.dram_tensor("b3v", [OUT, 1], F32, kind="ExternalInput")
    out_d = nc.dram_tensor("out", [OUT, BL], F32, kind="ExternalOutput")

    with TileContext(nc) as tc:
        with (
            tc.tile_pool(name="wp", bufs=1) as wp,
            tc.tile_pool(name="xp", bufs=XB) as xp,
            tc.tile_pool(name="hp", bufs=HB) as hp,
            tc.tile_pool(name="mp", bufs=MB) as mp,
            tc.tile_pool(name="op", bufs=1) as op,
            tc.tile_pool(name="ps", bufs=PB, space="PSUM") as psp,
            tc.tile_pool(name="ps3", bufs=1, space="PSUM") as ps3p,
        ):
            # ---- PE warm-up: dummy matmuls with no input deps fill the
            # initial DMA wait and lift the HAM clock gate to 2.4 GHz
            # before the first real matmul group ----
            warm_sb = wp.tile([128, 64], F32R, tag="warm")
            nc.sync.dma_start(out=warm_sb, in_=xT[0:128, 0:64])
            warm_ps = psp.tile([128, BP], F32, tag="ps")
            for _w in range(48):
                nc.tensor.matmul(
                    warm_ps[:G, :64], warm_sb[:, :G], warm_sb[:, :],
                    start=True, stop=True,
                )

            # ---- RNN weights + biases; only wi0 + bias0 gate the first
            # matmul group, the rest loads behind the first obs tiles ----
            w_rnn = {}
            b_rnn = []

            def load_rnn_w(l, key, drt):
                tw = wp.tile([128, NKH, H], F32R, tag=f"w{key}{l}")
                nc.sync.dma_start(
                    out=tw, in_=drt[:, :].rearrange("(ko ki) m -> ki ko m", ki=128)
                )
                w_rnn[(l, key)] = tw

            load_rnn_w(0, "i", wi_d[0])
            for l in range(2):
                tb = wp.tile([128, NKH], F32, tag=f"brnn{l}")
                nc.sync.dma_start(out=tb, in_=brnn_d[l][:, :])
                b_rnn.append(tb)

            for _rep in range(K_REPS):
                # ---- obs tiles for t=0 and goal (critical path first) ----
                x_tiles = {}  # (p, t, kb) -> tile [128, BP]

                def load_x(p, t, kb):
                    fr = (T - 1) - t  # time reversal
                    tile = xp.tile([128, BP], F32R, tag="x")
                    r0 = fr * H + kb * 128
                    if t == 0:
                        # split the critical first-step loads so the n=0
                        # matmul group can start after half a tile lands
                        # (Tile tracks subtile deps)
                        nc.sync.dma_start(
                            out=tile[:, 0:512],
                            in_=xT[r0 : r0 + 128, p * BP : p * BP + 512],
                        )
                        nc.sync.dma_start(
                            out=tile[:, 512:BP],
                            in_=xT[r0 : r0 + 128, p * BP + 512 : (p + 1) * BP],
                        )
                    else:
                        nc.sync.dma_start(
                            out=tile, in_=xT[r0 : r0 + 128, p * BP : (p + 1) * BP]
                        )
                    x_tiles[(p, t, kb)] = tile

                for p in range(NPASS):
                    for kb in range(NKH):
                        load_x(p, 0, kb)
                if _rep == 0:
                    load_rnn_w(0, "h", wh_d[0])
                    load_rnn_w(1, "i", wi_d[1])
                    load_rnn_w(1, "h", wh_d[1])
                for t in range(1, T):
                    for p in range(NPASS):
                        for kb in range(NKH):
                            load_x(p, t, kb)
                # goal is only needed by the MLP; load it after the obs tiles
                goal_sb = op.tile([G, BL], F32R, tag="goal")
                nc.sync.dma_start(out=goal_sb, in_=xT[OBS : OBS + G, :])

                if _rep == 0:
                    # ---- MLP weights + biases (after the first x loads so
                    # the RNN critical path starts immediately) ----
                    w1_sb = wp.tile([128, 3, HID], F32R, tag="w1")
                    nc.sync.dma_start(
                        out=w1_sb[:, 0:2, :],
                        in_=w1_d[0:H, :].rearrange("(ko ki) m -> ki ko m", ki=128),
                    )
                    nc.sync.dma_start(out=w1_sb[:G, 2, :], in_=w1_d[H : H + G, :])
                    w2_sb = wp.tile([128, 8, HID], F32R, tag="w2")
                    # split the 4MB load across the DMA queues: one 512KB
                    # chunk per k-block instead of 60us on a single queue
                    for ko in range(8):
                        nc.sync.dma_start(
                            out=w2_sb[:, ko, :],
                            in_=w2_d[ko * 128 : (ko + 1) * 128, :],
                        )
                    w3_sb = wp.tile([128, 8, OUT], F32R, tag="w3")
                    nc.sync.dma_start(
                        out=w3_sb,
                        in_=w3_d[:, :].rearrange("(ko ki) m -> ki ko m", ki=128),
                    )
                    b1_sb = wp.tile([128, 8], F32, tag="b1")
                    nc.sync.dma_start(out=b1_sb, in_=b1_d[:, :])
                    b2_sb = wp.tile([128, 8], F32, tag="b2")
                    nc.sync.dma_start(out=b2_sb, in_=b2_d[:, :])
                    b3_sb = wp.tile([G, 1], F32, tag="b3")
                    nc.sync.dma_start(out=b3_sb, in_=b3_d[:, :])

                # ---- RNN: 2 layers x 5 steps, both passes interleaved ----
                h_prev = {p: [None, None] for p in range(NPASS)}  # per layer: [m0, m1]
                h_cur = {p: [None, None] for p in range(NPASS)}
                for t in range(K_T if K_RNN else 0):
                    for l in range(2):
                        for p in range(NPASS):
                            if l == 0:
                                src = [x_tiles[(p, t, kb)] for kb in range(NKH)]
                            else:
                                src = h_cur[p][0]
                            prev = h_prev[p][l]
                            new = []
                            for m in range(NKH):
                                if p == 1 and m == 1:
                                    # borrow the MLP3 pool's (idle) slot as a
                                    # 4th rotating accumulation buffer
                                    ps = ps3p.tile([128, BP], F32, tag="ps3")
                                else:
                                    ps = psp.tile([128, BP], F32, tag="ps")
                                msl = slice(m * 128, (m + 1) * 128)
                                # issue-order: put the operands whose producers
                                # finished earliest first, so the PE absorbs
                                # the wait for the freshest dependency once,
                                # at the end of the group.
                                # l=0: x (DMA, oldest) then h_{t-1};
                                # l=1: h2_{t-1} (older) then h1_t (freshest).
                                xops = [
                                    (src[kb], w_rnn[(l, "i")][:, kb, msl])
                                    for kb in range(NKH)
                                ]
                                hops = (
                                    [
                                        (prev[kb], w_rnn[(l, "h")][:, kb, msl])
                                        for kb in range(NKH)
                                    ]
                                    if prev is not None
                                    else []
                                )
                                ops = xops + hops if l == 0 else hops + xops
                                nch = BP // 512
                                # emit both n-chunks' early ops before any
                                # late ops: one consolidated stall per tile
                                for i, (rhs_t, w_ap) in enumerate(ops):
                                    for n in range(nch):
                                        nsl = slice(n * 512, (n + 1) * 512)
                                        nc.tensor.matmul(
                                            ps[:, nsl],
                                            w_ap,
                                            rhs_t[:, nsl],
                                            start=(i == 0),
                                            stop=(i == len(ops) - 1),
                                        )
                                ht = hp.tile([128, BP], F32R, tag="h")
                                if os.environ.get("K_SPLIT_TANH", "0") == "1":
                                    for n2 in range(BP // 512):
                                        n2sl = slice(n2 * 512, (n2 + 1) * 512)
                                        nc.scalar.activation(
                                            ht[:, n2sl], ps[:, n2sl], TANH,
                                            bias=b_rnn[l][:, m : m + 1],
                                        )
                                else:
                                    nc.scalar.activation(
                                        ht[:, :], ps[:, :], TANH,
                                        bias=b_rnn[l][:, m : m + 1],
                                    )
                                new.append(ht)
                            h_cur[p][l] = new
                    for p in range(NPASS):
                        h_prev[p] = [h_cur[p][0], h_cur[p][1]]

                # ---- MLP ----
                out_sb = op.tile([OUT, BL], F32, tag="o")
                if not K_MLP:
                    nc.vector.memset(out_sb[:, :], 0.0)
                    nc.sync.dma_start(out=out_d[:, :], in_=out_sb)
                for p in range(NPASS if K_MLP else 0):
                    h2last = h_cur[p][1]
                    if h2last is None:  # RNN skipped: use raw x tiles instead
                        h2last = [x_tiles[(p, 0, kb)] for kb in range(NKH)]
                    # layer 1: [320 -> 1024]
                    h1_tiles = []
                    for m in range(8):
                        ps = psp.tile([128, BP], F32, tag="ps")
                        msl = slice(m * 128, (m + 1) * 128)
                        for n in range(BP // 512):
                            nsl = slice(n * 512, (n + 1) * 512)
                            gsl = slice(p * BP + n * 512, p * BP + (n + 1) * 512)
                            # goal block first: it has no RNN dependency, so
                            # it issues in the shadow of the RNN tail
                            ops = [(goal_sb[:, gsl], w1_sb[:G, 2, msl])] + [
                                (h2last[kb][:, nsl], w1_sb[:, kb, msl])
                                for kb in range(NKH)
                            ]
                            for i, (rhs_ap, w_ap) in enumerate(ops):
                                nc.tensor.matmul(
                                    ps[:, nsl],
                                    w_ap,
                                    rhs_ap,
                                    start=(i == 0),
                                    stop=(i == len(ops) - 1),
                                )
                        t1 = mp.tile([128, BP], F32R, tag="m")
                        nc.scalar.activation(
                            t1[:, :], ps[:, :], RELU, bias=b1_sb[:, m : m + 1]
                        )
                        h1_tiles.append(t1)
                    # layer 2: [1024 -> 1024], layer 3 accumulated incrementally
                    ps3 = ps3p.tile([G, BP], F32, tag="ps3")
                    for m in range(8):
                        ps = psp.tile([128, BP], F32, tag="ps")
                        msl = slice(m * 128, (m + 1) * 128)
                        for n in range(BP // 512):
                            nsl = slice(n * 512, (n + 1) * 512)
                            for k in range(8):
                                nc.tensor.matmul(
                                    ps[:, nsl],
                                    w2_sb[:, k, msl],
                                    h1_tiles[k][:, nsl],
                                    start=(k == 0),
                                    stop=(k == 7),
                                )
                        t2 = mp.tile([128, BP], F32R, tag="m")
                        nc.scalar.activation(
                            t2[:, :], ps[:, :], RELU, bias=b2_sb[:, m : m + 1]
                        )
                        for n in range(BP // 512):
                            nsl = slice(n * 512, (n + 1) * 512)
                            nc.tensor.matmul(
                                ps3[:, nsl],
                                w3_sb[:, m, :],
                                t2[:, nsl],
                                start=(m == 0),
                                stop=(m == 7),
                            )
                    nc.scalar.activation(
                        out_sb[:, p * BP : (p + 1) * BP], ps3[:, :], TANH, bias=b3_sb[:, 0:1]
                    )
                    # ship this pass's half immediately; overlaps the other
                    # pass's MLP instead of serializing at the kernel tail
                    nc.sync.dma_start(
                        out=out_d[:, p * BP : (p + 1) * BP],
                        in_=out_sb[:, p * BP : (p + 1) * BP],
                    )

    nc.finalize()
    return nc


class _Executor:
    """Compile a bass module once; run it on 8 cores via the axon PJRT path."""

    def __init__(self, build_fn=None):
        import jax
        from concourse import bass2jax
        import concourse.mybir as mybir

        self.jax = jax
        self.bass2jax = bass2jax
        bass2jax.install_neuronx_cc_hook()
        nc = (build_fn or _build_bass)()
        self.nc = nc

        self.partition_name = (
            nc.partition_id_tensor.name if nc.partition_id_tensor else None
        )
        in_names, out_names, out_avals = [], [], []
        for alloc in nc.m.functions[0].allocations:
            if not isinstance(alloc, mybir.MemoryLocationSet):
                continue
            name = alloc.memorylocations[0].name
            if alloc.kind == "ExternalInput":
                if name != self.partition_name:
                    in_names.append(name)
            elif alloc.kind == "ExternalOutput":
                out_names.append(name)
                out_avals.append(
                    jax.core.ShapedArray(tuple(alloc.tensor_shape), mybir.dt.np(alloc.dtype))
                )
        self.in_names = in_names
        self.out_names = out_names
        self.out_avals = out_avals
        # only the state slice differs per core; weights/biases are
        # replicated (transferred once, no 8x host-side concat)
        self.sharded_in = {"xT"}
        self._jitted = {}

    def _make(self, repeat):
        import jax
        from jax.experimental.shard_map import shard_map
        from jax.sharding import Mesh, PartitionSpec
        import numpy as np

        n_in = len(self.in_names)
        n_out = len(self.out_names)
        all_names = tuple(self.in_names) + tuple(self.out_names)
        if self.partition_name is not None:
            all_names = all_names + (self.partition_name,)
        nc = self.nc
        out_avals = tuple(self.out_avals)
        bass2jax = self.bass2jax

        def _body(*args):
            ins = list(args[:n_in])
            outs = list(args[n_in:])
            extra = (
                [bass2jax.partition_id_tensor()]
                if self.partition_name is not None
                else []
            )
            for _ in range(repeat):
                outs = list(
                    bass2jax._bass_exec_p.bind(
                        *ins,
                        *outs,
                        *extra,
                        out_avals=out_avals,
                        in_names=all_names,
                        out_names=tuple(self.out_names),
                        lowering_input_output_aliases=(),
                        sim_require_finite=True,
                        sim_require_nnan=True,
                        nc=nc,
                    )
                )
            return tuple(outs)

        devices = jax.devices()[:NCORES]
        mesh = Mesh(np.asarray(devices), ("core",))
        in_specs = tuple(
            PartitionSpec("core") if name in self.sharded_in else PartitionSpec()
            for name in self.in_names
        ) + (PartitionSpec("core"),) * n_out
        out_specs = (PartitionSpec("core"),) * n_out
        donate = tuple(range(n_in, n_in + n_out))
        return jax.jit(
            shard_map(_body, mesh=mesh, in_specs=in_specs, out_specs=out_specs,
                      check_rep=False),
            donate_argnums=donate,
            keep_unused=True,
        )

    def jitted(self, repeat=1):
        if repeat not in self._jitted:
            self._jitted[repeat] = self._make(repeat)
        return self._jitted[repeat]

    def concat_inputs(self, in_maps):
        """Global input arrays from per-core maps: sharded inputs concat
        along axis 0, replicated ones pass through from core 0."""
        out = []
        for name in self.in_names:
            if name in self.sharded_in:
                out.append(
                    np.concatenate([np.asarray(m[name]) for m in in_maps], axis=0)
                )
            else:
                out.append(np.asarray(in_maps[0][name]))
        return out

    def device_inputs(self, arrays):
        """device_put with the matching shardings (for repeated timed runs)."""
        import jax
        from jax.sharding import Mesh, NamedSharding, PartitionSpec

        mesh = Mesh(np.asarray(jax.devices()[:NCORES]), ("core",))
        out = []
        for name, a in zip(self.in_names, arrays):
            spec = PartitionSpec("core") if name in self.sharded_in else PartitionSpec()
            out.append(jax.device_put(a, NamedSharding(mesh, spec)))
        return out

    def zero_outs(self):
        return [
            np.zeros((NCORES * a.shape[0], *a.shape[1:]), a.dtype)
            for a in self.out_avals
        ]

    def run(self, in_maps, repeat=1):
        fn = self.jitted(repeat)
        arrays = []
        for name in self.in_names:
            if name == "xT" and "__xT_full" in in_maps[0]:
                arrays.append(in_maps[0]["__xT_full"])  # already concatenated
            elif name in self.sharded_in:
                arrays.append(
                    np.concatenate([np.asarray(m[name]) for m in in_maps], axis=0)
                )
            else:
                arrays.append(np.asarray(in_maps[0][name]))
        outs = fn(*arrays, *self.zero_outs())
        res = []
        for c in range(NCORES):
            res.append(
                {
                    name: np.asarray(outs[i]).reshape(NCORES, *self.out_avals[i].shape)[c]
                    for i, name in enumerate(self.out_names)
                }
            )
        return res


def _get_exec():
    """Executor for the grading path: always built with the full, single-rep
    network regardless of any debug env knobs."""
    global _EXEC
    if _EXEC is None:
        import os

        saved = {}
        for k in ("K_T", "K_MLP", "K_RNN", "K_REPS"):
            saved[k] = os.environ.pop(k, None)
        try:
            _EXEC = _Executor()
        finally:
            for k, v in saved.items():
                if v is not None:
                    os.environ[k] = v
    return _EXEC


def _prep_inputs(state, rnn_Wih, rnn_Whh, rnn_bih, rnn_bhh, W1, b1, W2, b2, W3, b3):
    f32 = np.float32
    state = np.asarray(state, f32)
    shared = {
        "wi0T": np.ascontiguousarray(np.asarray(rnn_Wih[0], f32).T),
        "wh0T": np.ascontiguousarray(np.asarray(rnn_Whh[0], f32).T),
        "wi1T": np.ascontiguousarray(np.asarray(rnn_Wih[1], f32).T),
        "wh1T": np.ascontiguousarray(np.asarray(rnn_Whh[1], f32).T),
        "brnn0": np.ascontiguousarray((np.asarray(rnn_bih[0], f32) + np.asarray(rnn_bhh[0], f32)).reshape(2, 128).T),
        "brnn1": np.ascontiguousarray((np.asarray(rnn_bih[1], f32) + np.asarray(rnn_bhh[1], f32)).reshape(2, 128).T),
        "w1T": np.ascontiguousarray(np.asarray(W1, f32).T),
        "w2T": np.ascontiguousarray(np.asarray(W2, f32).T),
        "w3T": np.ascontiguousarray(np.asarray(W3, f32).T),
        "b1v": np.ascontiguousarray(np.asarray(b1, f32).reshape(8, 128).T),
        "b2v": np.ascontiguousarray(np.asarray(b2, f32).reshape(8, 128).T),
        "b3v": np.asarray(b3, f32).reshape(OUT, 1),
    }
    # one fused reshape-transpose-copy: [B, F] -> [NCORES*F, BL] where each
    # core's block is its state slice transposed (feature-major)
    xT_full = np.ascontiguousarray(
        state.reshape(NCORES, BL, OBS + G).transpose(0, 2, 1)
    ).reshape(NCORES * (OBS + G), BL)
    F = OBS + G
    in_maps = []
    for c in range(NCORES):
        in_maps.append({"xT": xT_full[c * F : (c + 1) * F], **shared})
    in_maps[0]["__xT_full"] = xT_full  # fused array for the no-copy path
    return in_maps


def kernel(state, rnn_Wih, rnn_Whh, rnn_bih, rnn_bhh, W1, b1, W2, b2, W3, b3):
    global _EXEC
    in_maps = _prep_inputs(
        state, rnn_Wih, rnn_Whh, rnn_bih, rnn_bhh, W1, b1, W2, b2, W3, b3
    )
    try:
        res = _get_exec().run(in_maps)
    except Exception:
        # transient device faults (e.g. NRT exec-unit errors) have been seen
        # on this fabric; rebuild the executable once and retry
        _EXEC = None
        res = _get_exec().run(in_maps)
    return np.concatenate([res[c]["out"].T for c in range(NCORES)], axis=0)



# revision 26
# speedup vs baseline: 1.1740x; 1.1740x over previous
"""Trainium2 Bass kernel for DeterministicActorRNN.

Network (per sample):
  obs   = state[:, :1280] -> 5 frames of 256, time-reversed
  2-layer tanh RNN (H=256, T=5)  -> last hidden of layer 2
  MLP: relu(320->1024), relu(1024->1024), 64 <- tanh(1024->64)

Strategy: pure data parallel over 8 NeuronCores (2048 rows each).
All activations are kept feature-major ([feature, batch]) so every matmul
contracts the partition dimension; the host pre-transposes the state slice
and all weight matrices.  Weights and streamed activations are float16
(11-bit mantissa — the same multiply precision as float32r, at half the
bytes; fp16 weights also qualify for fast-weight-load, dropping the
matmul cadence from ~234 to ~218 ns for N=512).  PSUM accumulation and
all bias/tanh/relu math stay fp32.  Each core processes its 2048 rows as
2 independent batch-chunks of 1024 to bound SBUF usage and give the
scheduler two independent RNN chains to interleave.

Every weight tile is kept a uniform (128,128) PE tile — the MLP-L1 goal
block is K-padded from 64 to 128 with zeros and W3 is M-padded from 64
to 128 — because mixed tile sizes stall the LDWEIGHTS double-buffer
(~450ns per L1 triple, ~100ns around each L3 matmul, measured on HW).
Steady-state body period ~146.4us/core vs a 672-matmul stream floor of
~145us; the PE never idles between reps.
"""

import numpy as np

B = 16384
NCORES = 8
BL = B // NCORES          # 2048 rows per core
NPASS = 2
BP = BL // NPASS          # 1024 rows per pass
T = 5
H = 256                   # RNN hidden / obs per frame
G = 64                    # goal dim
OBS = T * H               # 1280
HID = 1024
OUT = 64
NKH = H // 128            # 2 feature blocks of 128

import os as _os
# stream dtype knob: f16 (default) | bf16 | f32r.  f16 keeps fp32r-grade
# matmul precision (11-bit mantissa) at half the operand bytes; weights
# qualify for fast-weight-load and the cadence drops ~234 -> ~226 ns/mm.
SDT = _os.environ.get("K_SDT", "f16")
if _os.environ.get("K_BF16", "0") == "1":
    SDT = "bf16"
BF16 = SDT != "f32r"

_EXEC = None              # compiled executable cache

# matmul-stream shape, used by test.py's NTFF-profile timing to find
# rep boundaries in the hardware trace
N_WARM_MM = 48            # one-time PE warm-up matmuls
N_MM_PER_REP = 672        # matmuls per body repetition


def _build_bass():
    import os
    import concourse.bass as bass  # noqa: F401
    from concourse import bacc
    import concourse.mybir as mybir
    from concourse.tile import TileContext

    K_T = int(os.environ.get("K_T", T))          # RNN steps to run
    K_MLP = int(os.environ.get("K_MLP", "1"))    # include MLP
    K_RNN = int(os.environ.get("K_RNN", "1"))    # include RNN
    K_REPS = int(os.environ.get("K_REPS", "1"))  # repeat whole body (timing)
    XB = int(os.environ.get("K_XB", "6"))        # x pool bufs
    HB = int(os.environ.get("K_HB", "10"))       # h pool bufs
    MB = int(os.environ.get("K_MB", "16"))       # mlp pool bufs
    PB = int(os.environ.get("K_PB", "3"))        # psum pool bufs

    F32 = mybir.dt.float32
    F32R = {"f32r": mybir.dt.float32r, "bf16": mybir.dt.bfloat16, "f16": mybir.dt.float16}[SDT]
    TANH = mybir.ActivationFunctionType.Tanh
    RELU = mybir.ActivationFunctionType.Relu

    nc = bacc.Bacc(None, target_bir_lowering=False)

    xT = nc.dram_tensor("xT", [OBS + G, BL], F32R, kind="ExternalInput")
    wi_d = [nc.dram_tensor(f"wi{l}T", [H, H], F32R, kind="ExternalInput") for l in range(2)]
    wh_d = [nc.dram_tensor(f"wh{l}T", [H, H], F32R, kind="ExternalInput") for l in range(2)]
    brnn_d = [nc.dram_tensor(f"brnn{l}", [128, NKH], F32, kind="ExternalInput") for l in range(2)]
    w1_d = nc.dram_tensor("w1T", [H + G, HID], F32R, kind="ExternalInput")
    w2_d = nc.dram_tensor("w2T", [HID, HID], F32R, kind="ExternalInput")
    w3_d = nc.dram_tensor("w3T", [HID, OUT], F32R, kind="ExternalInput")
    b1_d = nc.dram_tensor("b1v", [128, 8], F32, kind="ExternalInput")
    b2_d = nc.dram_tensor("b2v", [128, 8], F32, kind="ExternalInput")
    b3_d = nc.dram_tensor("b3v", [OUT, 1], F32, kind="ExternalInput")
    out_d = nc.dram_tensor("out", [OUT, BL], F32, kind="ExternalOutput")

    with TileContext(nc) as tc:
        with (
            tc.tile_pool(name="wp", bufs=1) as wp,
            tc.tile_pool(name="xp", bufs=XB) as xp,
            tc.tile_pool(name="hp", bufs=HB) as hp,
            tc.tile_pool(name="mp", bufs=MB) as mp,
            tc.tile_pool(name="op", bufs=1) as op,
            tc.tile_pool(name="ps", bufs=PB, space="PSUM") as psp,
            tc.tile_pool(name="ps3", bufs=1, space="PSUM") as ps3p,
        ):
            # ---- PE warm-up: dummy matmuls with no input deps fill the
            # initial DMA wait and lift the HAM clock gate to 2.4 GHz
            # before the first real matmul group ----
            warm_sb = wp.tile([128, 64], F32R, tag="warm")
            nc.sync.dma_start(out=warm_sb, in_=xT[0:128, 0:64])
            warm_ps = psp.tile([128, BP], F32, tag="ps")
            for _w in range(48):
                nc.tensor.matmul(
                    warm_ps[:G, :64], warm_sb[:, :G], warm_sb[:, :],
                    start=True, stop=True,
                )

            # ---- goal tile padded to K=128 (rows G..127 zero) so every
            # MLP-L1 matmul has a uniform (128,128) weight tile: mixed
            # 64/128 tile sizes break LDWEIGHTS pipelining (~450ns per
            # (m,n) triple measured) ----
            goal_sb = op.tile([128, BL], F32R, tag="goal")
            nc.vector.memset(goal_sb[G:, :] if BF16 else goal_sb[G:, :].bitcast(F32), 0.0)

            # ---- RNN weights + biases; only wi0 + bias0 gate the first
            # matmul group, the rest loads behind the first obs tiles ----
            w_rnn = {}
            b_rnn = []

            def load_rnn_w(l, key, drt):
                tw = wp.tile([128, NKH, H], F32R, tag=f"w{key}{l}")
                nc.sync.dma_start(
                    out=tw, in_=drt[:, :].rearrange("(ko ki) m -> ki ko m", ki=128)
                )
                w_rnn[(l, key)] = tw

            load_rnn_w(0, "i", wi_d[0])
            for l in range(2):
                tb = wp.tile([128, NKH], F32, tag=f"brnn{l}")
                nc.sync.dma_start(out=tb, in_=brnn_d[l][:, :])
                b_rnn.append(tb)

            for _rep in range(K_REPS):
                # ---- obs tiles for t=0 and goal (critical path first) ----
                x_tiles = {}  # (p, t, kb) -> tile [128, BP]

                def load_x(p, t, kb):
                    fr = (T - 1) - t  # time reversal
                    tile = xp.tile([128, BP], F32R, tag="x")
                    r0 = fr * H + kb * 128
                    if t == 0:
                        # split the critical first-step loads so the n=0
                        # matmul group can start after half a tile lands
                        # (Tile tracks subtile deps)
                        nc.sync.dma_start(
                            out=tile[:, 0:512],
                            in_=xT[r0 : r0 + 128, p * BP : p * BP + 512],
                        )
                        nc.sync.dma_start(
                            out=tile[:, 512:BP],
                            in_=xT[r0 : r0 + 128, p * BP + 512 : (p + 1) * BP],
                        )
                    else:
                        nc.sync.dma_start(
                            out=tile, in_=xT[r0 : r0 + 128, p * BP : (p + 1) * BP]
                        )
                    x_tiles[(p, t, kb)] = tile

                for p in range(NPASS):
                    for kb in range(NKH):
                        load_x(p, 0, kb)
                if _rep == 0:
                    load_rnn_w(0, "h", wh_d[0])
                    load_rnn_w(1, "i", wi_d[1])
                    load_rnn_w(1, "h", wh_d[1])
                for t in range(1, T):
                    for p in range(NPASS):
                        for kb in range(NKH):
                            load_x(p, t, kb)
                # goal is only needed by the MLP; load it after the obs tiles
                nc.sync.dma_start(out=goal_sb[:G, :], in_=xT[OBS : OBS + G, :])

                if _rep == 0:
                    # ---- MLP weights + biases (after the first x loads so
                    # the RNN critical path starts immediately) ----
                    w1_sb = wp.tile([128, 3, HID], F32R, tag="w1")
                    nc.sync.dma_start(
                        out=w1_sb[:, 0:2, :],
                        in_=w1_d[0:H, :].rearrange("(ko ki) m -> ki ko m", ki=128),
                    )
                    nc.sync.dma_start(out=w1_sb[:G, 2, :], in_=w1_d[H : H + G, :])
                    nc.vector.memset(w1_sb[G:, 2, :] if BF16 else w1_sb[G:, 2, :].bitcast(F32), 0.0)
                    w2_sb = wp.tile([128, 8, HID], F32R, tag="w2")
                    # split the 4MB load across the DMA queues: one 512KB
                    # chunk per k-block instead of 60us on a single queue
                    for ko in range(8):
                        nc.sync.dma_start(
                            out=w2_sb[:, ko, :],
                            in_=w2_d[ko * 128 : (ko + 1) * 128, :],
                        )
                    # W3 padded to M=128 (cols OUT..127 zero): keeps every
                    # weight tile a uniform (128,128) so the (128,64)<->
                    # (128,128) tile-size switches around each L3 matmul
                    # don't stall the LDWEIGHTS pipeline (~100ns x2 per
                    # m-block measured)
                    w3_sb = wp.tile([128, 8, 128], F32R, tag="w3")
                    nc.sync.dma_start(
                        out=w3_sb[:, :, 0:OUT],
                        in_=w3_d[:, :].rearrange("(ko ki) m -> ki ko m", ki=128),
                    )
                    nc.vector.memset(w3_sb[:, :, OUT:128] if BF16 else w3_sb[:, :, OUT:128].bitcast(F32), 0.0)
                    b1_sb = wp.tile([128, 8], F32, tag="b1")
                    nc.sync.dma_start(out=b1_sb, in_=b1_d[:, :])
                    b2_sb = wp.tile([128, 8], F32, tag="b2")
                    nc.sync.dma_start(out=b2_sb, in_=b2_d[:, :])
                    b3_sb = wp.tile([G, 1], F32, tag="b3")
                    nc.sync.dma_start(out=b3_sb, in_=b3_d[:, :])

                # ---- RNN: 2 layers x 5 steps, both passes interleaved ----
                h_prev = {p: [None, None] for p in range(NPASS)}  # per layer: [m0, m1]
                h_cur = {p: [None, None] for p in range(NPASS)}
                for t in range(K_T if K_RNN else 0):
                    for l in range(2):
                        for p in range(NPASS):
                            if l == 0:
                                src = [x_tiles[(p, t, kb)] for kb in range(NKH)]
                            else:
                                src = h_cur[p][0]
                            prev = h_prev[p][l]
                            new = []
                            for m in range(NKH):
                                if p == 1 and m == 1:
                                    # borrow the MLP3 pool's (idle) slot as a
                                    # 4th rotating accumulation buffer
                                    ps = ps3p.tile([128, BP], F32, tag="ps3")
                                else:
                                    ps = psp.tile([128, BP], F32, tag="ps")
                                msl = slice(m * 128, (m + 1) * 128)
                                # issue-order: put the operands whose producers
                                # finished earliest first, so the PE absorbs
                                # the wait for the freshest dependency once,
                                # at the end of the group.
                                # l=0: x (DMA, oldest) then h_{t-1};
                                # l=1: h2_{t-1} (older) then h1_t (freshest).
                                xops = [
                                    (src[kb], w_rnn[(l, "i")][:, kb, msl])
                                    for kb in range(NKH)
                                ]
                                hops = (
                                    [
                                        (prev[kb], w_rnn[(l, "h")][:, kb, msl])
                                        for kb in range(NKH)
                                    ]
                                    if prev is not None
                                    else []
                                )
                                ops = xops + hops if l == 0 else hops + xops
                                nch = BP // 512
                                # emit both n-chunks' early ops before any
                                # late ops: one consolidated stall per tile
                                for i, (rhs_t, w_ap) in enumerate(ops):
                                    for n in range(nch):
                                        nsl = slice(n * 512, (n + 1) * 512)
                                        nc.tensor.matmul(
                                            ps[:, nsl],
                                            w_ap,
                                            rhs_t[:, nsl],
                                            start=(i == 0),
                                            stop=(i == len(ops) - 1),
                                        )
                                ht = hp.tile([128, BP], F32R, tag="h")
                                if os.environ.get("K_SPLIT_TANH", "0") == "1":
                                    for n2 in range(BP // 512):
                                        n2sl = slice(n2 * 512, (n2 + 1) * 512)
                                        nc.scalar.activation(
                                            ht[:, n2sl], ps[:, n2sl], TANH,
                                            bias=b_rnn[l][:, m : m + 1],
                                        )
                                else:
                                    nc.scalar.activation(
                                        ht[:, :], ps[:, :], TANH,
                                        bias=b_rnn[l][:, m : m + 1],
                                    )
                                new.append(ht)
                            h_cur[p][l] = new
                    for p in range(NPASS):
                        h_prev[p] = [h_cur[p][0], h_cur[p][1]]

                # ---- MLP ----
                out_sb = op.tile([OUT, BL], F32, tag="o")
                if not K_MLP:
                    nc.vector.memset(out_sb[:, :], 0.0)
                    nc.sync.dma_start(out=out_d[:, :], in_=out_sb)
                for p in range(NPASS if K_MLP else 0):
                    h2last = h_cur[p][1]
                    if h2last is None:  # RNN skipped: use raw x tiles instead
                        h2last = [x_tiles[(p, 0, kb)] for kb in range(NKH)]
                    # layer 1: [320 -> 1024]
                    h1_tiles = []
                    for m in range(8):
                        ps = psp.tile([128, BP], F32, tag="ps")
                        msl = slice(m * 128, (m + 1) * 128)
                        for n in range(BP // 512):
                            nsl = slice(n * 512, (n + 1) * 512)
                            gsl = slice(p * BP + n * 512, p * BP + (n + 1) * 512)
                            # goal block first: it has no RNN dependency, so
                            # it issues in the shadow of the RNN tail
                            ops = [(goal_sb[:, gsl], w1_sb[:, 2, msl])] + [
                                (h2last[kb][:, nsl], w1_sb[:, kb, msl])
                                for kb in range(NKH)
                            ]
                            for i, (rhs_ap, w_ap) in enumerate(ops):
                                nc.tensor.matmul(
                                    ps[:, nsl],
                                    w_ap,
                                    rhs_ap,
                                    start=(i == 0),
                                    stop=(i == len(ops) - 1),
                                )
                        t1 = mp.tile([128, BP], F32R, tag="m")
                        nc.scalar.activation(
                            t1[:, :], ps[:, :], RELU, bias=b1_sb[:, m : m + 1]
                        )
                        h1_tiles.append(t1)
                    # layer 2: [1024 -> 1024], layer 3 accumulated incrementally
                    # (rows OUT..127 of ps3 receive the zero-padded W3 columns)
                    ps3 = ps3p.tile([128, BP], F32, tag="ps3")
                    for m in range(8):
                        ps = psp.tile([128, BP], F32, tag="ps")
                        msl = slice(m * 128, (m + 1) * 128)
                        for n in range(BP // 512):
                            nsl = slice(n * 512, (n + 1) * 512)
                            for k in range(8):
                                nc.tensor.matmul(
                                    ps[:, nsl],
                                    w2_sb[:, k, msl],
                                    h1_tiles[k][:, nsl],
                                    start=(k == 0),
                                    stop=(k == 7),
                                )
                        t2 = mp.tile([128, BP], F32R, tag="m")
                        nc.scalar.activation(
                            t2[:, :], ps[:, :], RELU, bias=b2_sb[:, m : m + 1]
                        )
                        for n in range(BP // 512):
                            nsl = slice(n * 512, (n + 1) * 512)
                            nc.tensor.matmul(
                                ps3[:, nsl],
                                w3_sb[:, m, :],
                                t2[:, nsl],
                                start=(m == 0),
                                stop=(m == 7),
                            )
                    nc.scalar.activation(
                        out_sb[:, p * BP : (p + 1) * BP], ps3[:G, :], TANH, bias=b3_sb[:, 0:1]
                    )
                    # ship this pass's half immediately; overlaps the other
                    # pass's MLP instead of serializing at the kernel tail
                    nc.sync.dma_start(
                        out=out_d[:, p * BP : (p + 1) * BP],
                        in_=out_sb[:, p * BP : (p + 1) * BP],
                    )

    nc.finalize()
    return nc


class _Executor:
    """Compile a bass module once; run it on 8 cores via the axon PJRT path."""

    def __init__(self, build_fn=None):
        import jax
        from concourse import bass2jax
        import concourse.mybir as mybir

        self.jax = jax
        self.bass2jax = bass2jax
        bass2jax.install_neuronx_cc_hook()
        nc = (build_fn or _build_bass)()
        self.nc = nc

        self.partition_name = (
            nc.partition_id_tensor.name if nc.partition_id_tensor else None
        )
        in_names, out_names, out_avals = [], [], []
        for alloc in nc.m.functions[0].allocations:
            if not isinstance(alloc, mybir.MemoryLocationSet):
                continue
            name = alloc.memorylocations[0].name
            if alloc.kind == "ExternalInput":
                if name != self.partition_name:
                    in_names.append(name)
            elif alloc.kind == "ExternalOutput":
                out_names.append(name)
                out_avals.append(
                    jax.core.ShapedArray(tuple(alloc.tensor_shape), mybir.dt.np(alloc.dtype))
                )
        self.in_names = in_names
        self.out_names = out_names
        self.out_avals = out_avals
        # only the state slice differs per core; weights/biases are
        # replicated (transferred once, no 8x host-side concat)
        self.sharded_in = {"xT"}
        self._jitted = {}

    def _make(self, repeat):
        import jax
        from jax.experimental.shard_map import shard_map
        from jax.sharding import Mesh, PartitionSpec
        import numpy as np

        n_in = len(self.in_names)
        n_out = len(self.out_names)
        all_names = tuple(self.in_names) + tuple(self.out_names)
        if self.partition_name is not None:
            all_names = all_names + (self.partition_name,)
        nc = self.nc
        out_avals = tuple(self.out_avals)
        bass2jax = self.bass2jax

        def _body(*args):
            ins = list(args[:n_in])
            outs = list(args[n_in:])
            extra = (
                [bass2jax.partition_id_tensor()]
                if self.partition_name is not None
                else []
            )
            for _ in range(repeat):
                outs = list(
                    bass2jax._bass_exec_p.bind(
                        *ins,
                        *outs,
                        *extra,
                        out_avals=out_avals,
                        in_names=all_names,
                        out_names=tuple(self.out_names),
                        lowering_input_output_aliases=(),
                        sim_require_finite=True,
                        sim_require_nnan=True,
                        nc=nc,
                    )
                )
            return tuple(outs)

        devices = jax.devices()[:NCORES]
        mesh = Mesh(np.asarray(devices), ("core",))
        in_specs = tuple(
            PartitionSpec("core") if name in self.sharded_in else PartitionSpec()
            for name in self.in_names
        ) + (PartitionSpec("core"),) * n_out
        out_specs = (PartitionSpec("core"),) * n_out
        donate = tuple(range(n_in, n_in + n_out))
        return jax.jit(
            shard_map(_body, mesh=mesh, in_specs=in_specs, out_specs=out_specs,
                      check_rep=False),
            donate_argnums=donate,
            keep_unused=True,
        )

    def jitted(self, repeat=1):
        if repeat not in self._jitted:
            self._jitted[repeat] = self._make(repeat)
        return self._jitted[repeat]

    def concat_inputs(self, in_maps):
        """Global input arrays from per-core maps: sharded inputs concat
        along axis 0, replicated ones pass through from core 0."""
        out = []
        for name in self.in_names:
            if name in self.sharded_in:
                out.append(
                    np.concatenate([np.asarray(m[name]) for m in in_maps], axis=0)
                )
            else:
                out.append(np.asarray(in_maps[0][name]))
        return out

    def device_inputs(self, arrays):
        """device_put with the matching shardings (for repeated timed runs)."""
        import jax
        from jax.sharding import Mesh, NamedSharding, PartitionSpec

        mesh = Mesh(np.asarray(jax.devices()[:NCORES]), ("core",))
        out = []
        for name, a in zip(self.in_names, arrays):
            spec = PartitionSpec("core") if name in self.sharded_in else PartitionSpec()
            out.append(jax.device_put(a, NamedSharding(mesh, spec)))
        return out

    def zero_outs(self):
        return [
            np.zeros((NCORES * a.shape[0], *a.shape[1:]), a.dtype)
            for a in self.out_avals
        ]

    def run(self, in_maps, repeat=1):
        fn = self.jitted(repeat)
        arrays = []
        for name in self.in_names:
            if name == "xT" and "__xT_full" in in_maps[0]:
                arrays.append(in_maps[0]["__xT_full"])  # already concatenated
            elif name in self.sharded_in:
                arrays.append(
                    np.concatenate([np.asarray(m[name]) for m in in_maps], axis=0)
                )
            else:
                arrays.append(np.asarray(in_maps[0][name]))
        outs = fn(*arrays, *self.zero_outs())
        res = []
        for c in range(NCORES):
            res.append(
                {
                    name: np.asarray(outs[i]).reshape(NCORES, *self.out_avals[i].shape)[c]
                    for i, name in enumerate(self.out_names)
                }
            )
        return res


def _get_exec():
    """Executor for the grading path: always built with the full, single-rep
    network regardless of any debug env knobs."""
    global _EXEC
    if _EXEC is None:
        import os

        saved = {}
        for k in ("K_T", "K_MLP", "K_RNN", "K_REPS"):
            saved[k] = os.environ.pop(k, None)
        try:
            _EXEC = _Executor()
        finally:
            for k, v in saved.items():
                if v is not None:
                    os.environ[k] = v
    return _EXEC


def _prep_inputs(state, rnn_Wih, rnn_Whh, rnn_bih, rnn_bhh, W1, b1, W2, b2, W3, b3):
    f32 = np.float32
    if SDT == "bf16":
        import ml_dtypes
        sdt = ml_dtypes.bfloat16
    elif SDT == "f16":
        sdt = np.float16
    else:
        sdt = f32
    state = np.asarray(state, f32)
    shared = {
        "wi0T": np.ascontiguousarray(np.asarray(rnn_Wih[0], f32).T.astype(sdt)),
        "wh0T": np.ascontiguousarray(np.asarray(rnn_Whh[0], f32).T.astype(sdt)),
        "wi1T": np.ascontiguousarray(np.asarray(rnn_Wih[1], f32).T.astype(sdt)),
        "wh1T": np.ascontiguousarray(np.asarray(rnn_Whh[1], f32).T.astype(sdt)),
        "brnn0": np.ascontiguousarray((np.asarray(rnn_bih[0], f32) + np.asarray(rnn_bhh[0], f32)).reshape(2, 128).T),
        "brnn1": np.ascontiguousarray((np.asarray(rnn_bih[1], f32) + np.asarray(rnn_bhh[1], f32)).reshape(2, 128).T),
        "w1T": np.ascontiguousarray(np.asarray(W1, f32).T.astype(sdt)),
        "w2T": np.ascontiguousarray(np.asarray(W2, f32).T.astype(sdt)),
        "w3T": np.ascontiguousarray(np.asarray(W3, f32).T.astype(sdt)),
        "b1v": np.ascontiguousarray(np.asarray(b1, f32).reshape(8, 128).T),
        "b2v": np.ascontiguousarray(np.asarray(b2, f32).reshape(8, 128).T),
        "b3v": np.asarray(b3, f32).reshape(OUT, 1),
    }
    # one fused reshape-transpose-copy: [B, F] -> [NCORES*F, BL] where each
    # core's block is its state slice transposed (feature-major)
    xT_full = np.ascontiguousarray(
        state.reshape(NCORES, BL, OBS + G).transpose(0, 2, 1).astype(sdt)
    ).reshape(NCORES * (OBS + G), BL)
    F = OBS + G
    in_maps = []
    for c in range(NCORES):
        in_maps.append({"xT": xT_full[c * F : (c + 1) * F], **shared})
    in_maps[0]["__xT_full"] = xT_full  # fused array for the no-copy path
    return in_maps


def kernel(state, rnn_Wih, rnn_Whh, rnn_bih, rnn_bhh, W1, b1, W2, b2, W3, b3):
    global _EXEC
    in_maps = _prep_inputs(
        state, rnn_Wih, rnn_Whh, rnn_bih, rnn_bhh, W1, b1, W2, b2, W3, b3
    )
    try:
        res = _get_exec().run(in_maps)
    except Exception:
        # transient device faults (e.g. NRT exec-unit errors) have been seen
        # on this fabric; rebuild the executable once and retry
        _EXEC = None
        res = _get_exec().run(in_maps)
    return np.concatenate([res[c]["out"].T for c in range(NCORES)], axis=0)

